# revision 5
# baseline (speedup 1.0000x reference)
"""Trainium2 Bass kernel for Chn8ActGrp3WgtQuantizedLinear.

Computes: out = fake_quant8_per_row(x) @ dequant(weight_qvals, weight_scales).T

  x:             (1024, 4096)  f32
  weight_qvals:  (11008, 4096) int32, 3-bit values in [-4, 3]
  weight_scales: (11008, 32)   f32, one scale per (out-channel, 128-group)
  out:           (1024, 11008) f32

Strategy (tensor parallel over 8 NeuronCores):
  - shard N=11008 output channels -> 1376 per core; replicate x
  - host repacks weights K-major: wq[4096,1376] bf16 (values exact in bf16),
    ws[32,1376] f32
  - device per core:
      * dequant W[k,n] = wq * ws[g(k),n] -> bf16 (gpsimd partition_broadcast
        of the scale row + DVE tensor_tensor)
      * per-row activation stats (min/max), scale/zero-point; fake-quant via
        one ACT op (x*inv_s + MAGIC rounds to integer) + clip; a = qx - zero
        is integer in [-255,255], exact in bf16
      * transpose a via PE transpose-mode into aT[k,m] tiles
      * matmul: psum[m=128, n] += aT[k,m].T @ W[k,n] over 32 k-groups
      * evict with per-row scale: out = psum * scale[m]
  - host concatenates the 8 (1024, 1376) shards.
"""

import os
import sys
import types

import numpy as np
import ml_dtypes

M, K, N, GS = 1024, 4096, 11008, 128
NCORES = 8
NC_SHARD = N // NCORES  # 1376
NGRP = K // GS  # 32
MTILES = M // 128  # 8
MAGIC = 12582912.0  # 1.5 * 2**23: adding then subtracting rounds f32 to int (RNE)

_CACHE = {}
LAST_RESULTS = None


def _install_axon_ntff_hook():
    """Register the NTFF profile hook if the container's antenv lacks it.

    Only needed for trace=True (BASS_TRACE=1); degrades silently."""
    try:
        if "antenv.axon_hooks" in sys.modules:
            return
        import antenv

        mod = types.ModuleType("antenv.axon_hooks")
        _state = {"hook": None}
        mod.set_axon_ntff_profile_hook = lambda h: _state.__setitem__("hook", h)
        mod.get_axon_ntff_profile_hook = lambda: _state["hook"]
        sys.modules["antenv.axon_hooks"] = mod
        antenv.axon_hooks = mod

        from trn_agent_boot.trn_boot import _ntff_profile_via_ctypes

        mod.set_axon_ntff_profile_hook(
            _ntff_profile_via_ctypes("/opt/axon/libaxon_pjrt.so")
        )
    except Exception:
        pass


def _build():
    if "nc" in _CACHE:
        return _CACHE["nc"]

    import concourse.bass as bass
    import concourse.tile as tile
    from concourse import bacc, mybir
    from concourse.masks import make_identity

    dt = mybir.dt
    F32, BF16 = dt.float32, dt.bfloat16
    ALU = mybir.AluOpType
    ACTF = mybir.ActivationFunctionType
    AX = mybir.AxisListType

    nc = bacc.Bacc("TRN2", target_bir_lowering=False, debug=False,
                   num_devices=NCORES)

    x_d = nc.dram_tensor("x", [M, K], F32, kind="ExternalInput").ap()
    wq_d = nc.dram_tensor("wq", [K, NC_SHARD], BF16, kind="ExternalInput").ap()
    ws_d = nc.dram_tensor("ws", [NGRP, NC_SHARD], F32, kind="ExternalInput").ap()
    out_d = nc.dram_tensor("out", [M, NC_SHARD], F32, kind="ExternalOutput").ap()

    # psum free-dim chunks (bank = 512 f32)
    CHUNKS = [(c, min(512, NC_SHARD - c)) for c in range(0, NC_SHARD, 512)]

    with tile.TileContext(nc) as tc:
        import contextlib

        ctx = contextlib.ExitStack()
        with ctx:
            consts = ctx.enter_context(tc.tile_pool(name="consts", bufs=1))
            wpool = ctx.enter_context(tc.tile_pool(name="w", bufs=1))
            wqld = ctx.enter_context(tc.tile_pool(name="wqld", bufs=2))
            wsp = ctx.enter_context(tc.tile_pool(name="ws", bufs=2))
            wsb = ctx.enter_context(tc.tile_pool(name="wsb", bufs=2))
            xp = ctx.enter_context(tc.tile_pool(name="x", bufs=2))
            up = ctx.enter_context(tc.tile_pool(name="u", bufs=1))
            ap_ = ctx.enter_context(tc.tile_pool(name="a", bufs=2))
            atp = ctx.enter_context(tc.tile_pool(name="at", bufs=2))
            outp = ctx.enter_context(tc.tile_pool(name="o", bufs=2))
            vecs = ctx.enter_context(tc.tile_pool(name="v", bufs=2))
            ps_out = ctx.enter_context(
                tc.tile_pool(name="pso", bufs=2, space="PSUM"))
            ps_tr = ctx.enter_context(
                tc.tile_pool(name="pst", bufs=2, space="PSUM"))

            ident = consts.tile([128, 128], BF16)
            make_identity(nc, ident[:])
            magic_vec = consts.tile([128, 1], F32)
            nc.vector.memset(magic_vec[:], MAGIC)



            # W holds all dequantized weights, k-major: [k%128, g, n]
            W = wpool.tile([128, NGRP * NC_SHARD], BF16)

            def wslice(g, c0, cw):
                return W[:, g * NC_SHARD + c0: g * NC_SHARD + c0 + cw]

            # ---- per-m-tile phases ----
            x_tiles = {}
            scale_of = {}
            at_of = {}

            def quant_phase(m):
                x_t = xp.tile([128, K], F32, tag="xt")
                nc.sync.dma_start(x_t[:], x_d[m * 128:(m + 1) * 128, :])
                mx = vecs.tile([128, 1], F32, tag="mx")
                nc.vector.tensor_reduce(mx[:], x_t[:], axis=AX.X, op=ALU.max)
                mn = vecs.tile([128, 1], F32, tag="mn")
                nc.vector.tensor_reduce(mn[:], x_t[:], axis=AX.X, op=ALU.min)
                xc = vecs.tile([128, 1], F32, tag="xc")
                nc.vector.tensor_scalar(xc[:], mx[:], 0.0, None, ALU.max)
                nn_ = vecs.tile([128, 1], F32, tag="nn")
                nc.vector.tensor_scalar(nn_[:], mn[:], 0.0, None, ALU.min)
                df = vecs.tile([128, 1], F32, tag="df")
                nc.vector.tensor_tensor(df[:], xc[:], nn_[:], ALU.subtract)
                sc = vecs.tile([128, 1], F32, tag="sc")
                nc.vector.tensor_scalar(sc[:], df[:], 1.0 / 255.0, 1e-9,
                                        ALU.mult, ALU.max)
                inv = vecs.tile([128, 1], F32, tag="inv")
                nc.vector.reciprocal(inv[:], sc[:])
                z0 = vecs.tile([128, 1], F32, tag="z0")
                nc.vector.tensor_tensor(z0[:], nn_[:], inv[:], ALU.mult)
                z1 = vecs.tile([128, 1], F32, tag="z1")
                nc.vector.tensor_scalar(z1[:], z0[:], -1.0, -128.0,
                                        ALU.mult, ALU.add)
                zr = vecs.tile([128, 1], F32, tag="zr")
                nc.vector.tensor_scalar(zr[:], z1[:], MAGIC, MAGIC,
                                        ALU.add, ALU.subtract)
                loM = vecs.tile([128, 1], F32, tag="loM")
                nc.vector.tensor_scalar(loM[:], zr[:], -1.0, MAGIC - 128.0,
                                        ALU.mult, ALU.add)
                hiM = vecs.tile([128, 1], F32, tag="hiM")
                nc.vector.tensor_scalar(hiM[:], zr[:], -1.0, MAGIC + 127.0,
                                        ALU.mult, ALU.add)
                # u = x*inv + MAGIC  (ACT; the add rounds to integer, RNE)
                u = up.tile([128, K], F32, tag="u")
                nc.scalar.activation(u[:], x_t[:], ACTF.Identity,
                                     bias=magic_vec[:], scale=inv[:])
                # clip in magic space (gpsimd; frees DVE)
                nc.gpsimd.tensor_scalar(u[:], u[:], loM[:], hiM[:],
                                        ALU.max, ALU.min)
                # a = u - MAGIC -> bf16 (integer in [-255, 255], exact)
                a_t = ap_.tile([128, K], BF16, tag="a")
                nc.vector.tensor_scalar(a_t[:], u[:], MAGIC, None, ALU.subtract)
                x_tiles[m] = x_t
                scale_of[m] = sc
                at_of[m] = a_t

            def transpose_phase(m):
                a_t = at_of[m]
                aT = atp.tile([128, NGRP * 128], BF16, tag="aT")
                for q in range(NGRP // 4):
                    tr = ps_tr.tile([128, 512], BF16, tag="tr")
                    for j in range(4):
                        g = q * 4 + j
                        nc.tensor.transpose(
                            tr[:, j * 128:(j + 1) * 128],
                            a_t[:, g * 128:(g + 1) * 128], ident[:])
                    nc.scalar.copy(aT[:, q * 512:(q + 1) * 512], tr[:])
                return aT

            def matmul_phase(m, aT):
                psum = ps_out.tile([128, NC_SHARD], F32, tag="psum")
                for g in range(NGRP):
                    for (c0, cw) in CHUNKS:
                        nc.tensor.matmul(
                            psum[:, c0:c0 + cw],
                            lhsT=aT[:, g * 128:(g + 1) * 128],
                            rhs=wslice(g, c0, cw),
                            start=(g == 0), stop=(g == NGRP - 1))
                o_t = outp.tile([128, NC_SHARD], F32, tag="o")
                nc.scalar.activation(o_t[:], psum[:], ACTF.Identity,
                                     bias=0.0, scale=scale_of[m][:])
                nc.sync.dma_start(out_d[m * 128:(m + 1) * 128, :], o_t[:])

            # Emission order tuned for pipeline fill: quant m0/m1 first so
            # DVE/ACT produce aT early, then weight dequant streams, then
            # the PE phases.
            quant_phase(0)
            quant_phase(1)

            for g in range(NGRP):
                wq_t = wqld.tile([128, NC_SHARD], BF16, tag="wq")
                nc.sync.dma_start(wq_t[:], wq_d[g * 128:(g + 1) * 128, :])
                ws_row = wsp.tile([1, NC_SHARD], F32, tag="wsrow")
                nc.sync.dma_start(ws_row[:], ws_d[g:g + 1, :])
                ws_bc = wsb.tile([128, NC_SHARD], F32, tag="wsb")
                nc.gpsimd.partition_broadcast(ws_bc[:], ws_row[:])
                nc.vector.tensor_tensor(W[:, g * NC_SHARD:(g + 1) * NC_SHARD],
                                        wq_t[:], ws_bc[:], ALU.mult)

            for m in range(MTILES):
                if m >= 2:
                    quant_phase(m)
                aT = transpose_phase(m)
                matmul_phase(m, aT)

    nc.compile()
    _CACHE["nc"] = nc
    return nc


def kernel(x, weight_qvals, weight_scales, group_size):
    global LAST_RESULTS
    _install_axon_ntff_hook()
    from concourse.bass_utils import run_bass_kernel_spmd

    x = np.asarray(x, dtype=np.float32)
    wq = np.asarray(weight_qvals)
    ws = np.asarray(weight_scales, dtype=np.float32)
    assert int(group_size) == GS
    assert x.shape == (M, K) and wq.shape == (N, K) and ws.shape == (N, NGRP)

    nc = _build()

    in_maps = []
    for c in range(NCORES):
        sl = slice(c * NC_SHARD, (c + 1) * NC_SHARD)
        wq_c = np.ascontiguousarray(wq[sl].T).astype(ml_dtypes.bfloat16)
        ws_c = np.ascontiguousarray(ws[sl].T)
        in_maps.append({"x": x, "wq": wq_c, "ws": ws_c})

    res = run_bass_kernel_spmd(nc, in_maps, core_ids=list(range(NCORES)))
    LAST_RESULTS = res
    out = np.concatenate([r["out"] for r in res.results], axis=1)
    return out


if __name__ == "__main__":
    rng = np.random.default_rng(0)
    xv = rng.standard_normal((M, K)).astype(np.float32)
    wqv = rng.integers(-4, 4, (N, K)).astype(np.int32)
    wsv = (rng.random((N, NGRP)).astype(np.float32) * 0.02 + 1e-4)
    o = kernel(xv, wqv, wsv, GS)
    print("out shape:", o.shape, "finite:", np.isfinite(o).all())


# revision 10
# speedup vs baseline: 2.0583x; 2.0583x over previous
"""Trainium2 Bass kernel for Chn8ActGrp3WgtQuantizedLinear.

Computes: out = fake_quant8_per_row(x) @ dequant(weight_qvals, weight_scales).T

  x:             (1024, 4096)  f32
  weight_qvals:  (11008, 4096) int32, 3-bit values in [-4, 3]
  weight_scales: (11008, 32)   f32, one scale per (out-channel, 128-group)
  out:           (1024, 11008) f32

Strategy (tensor parallel over 8 NeuronCores):
  - shard N=11008 output channels -> 1376 per core; replicate x
  - host repacks weights K-major: wq[4096,1376] bf16 (values exact in bf16),
    ws[32,1376] f32
  - device per core:
      * dequant W[k,n] = wq * ws[g(k),n] -> bf16 (gpsimd partition_broadcast
        of the scale row + DVE tensor_tensor)
      * per-row activation stats (min/max), scale/zero-point; fake-quant via
        one ACT op (x*inv_s + MAGIC rounds to integer) + clip; a = qx - zero
        is integer in [-255,255], exact in bf16
      * transpose a via PE transpose-mode into aT[k,m] tiles
      * matmul: psum[m=128, n] += aT[k,m].T @ W[k,n] over 32 k-groups
      * evict with per-row scale: out = psum * scale[m]
  - host concatenates the 8 (1024, 1376) shards.
"""

import os
import sys
import types

import numpy as np
import ml_dtypes

M, K, N, GS = 1024, 4096, 11008, 128
NCORES = 8
NC_SHARD = N // NCORES  # 1376
NGRP = K // GS  # 32
MTILES = M // 128  # 8
MAGIC = 12582912.0  # 1.5 * 2**23: adding then subtracting rounds f32 to int (RNE)

_CACHE = {}
LAST_RESULTS = None


def _install_axon_ntff_hook():
    """Register the NTFF profile hook if the container's antenv lacks it.

    Only needed for trace=True (BASS_TRACE=1); degrades silently."""
    try:
        if "antenv.axon_hooks" in sys.modules:
            return
        import antenv

        mod = types.ModuleType("antenv.axon_hooks")
        _state = {"hook": None}
        mod.set_axon_ntff_profile_hook = lambda h: _state.__setitem__("hook", h)
        mod.get_axon_ntff_profile_hook = lambda: _state["hook"]
        sys.modules["antenv.axon_hooks"] = mod
        antenv.axon_hooks = mod

        from trn_agent_boot.trn_boot import _ntff_profile_via_ctypes

        mod.set_axon_ntff_profile_hook(
            _ntff_profile_via_ctypes("/opt/axon/libaxon_pjrt.so")
        )
    except Exception:
        pass


def _build():
    if "nc" in _CACHE:
        return _CACHE["nc"]

    import concourse.bass as bass
    import concourse.tile as tile
    from concourse import bacc, mybir
    from concourse.masks import make_identity

    dt = mybir.dt
    F32, BF16 = dt.float32, dt.bfloat16
    ALU = mybir.AluOpType
    ACTF = mybir.ActivationFunctionType
    AX = mybir.AxisListType

    nc = bacc.Bacc("TRN2", target_bir_lowering=False, debug=False,
                   num_devices=NCORES)

    x_d = nc.dram_tensor("x", [M, K], F32, kind="ExternalInput").ap()
    wq_d = nc.dram_tensor("wq", [K, NC_SHARD], F32, kind="ExternalInput").ap()
    ws_d = nc.dram_tensor("ws", [NGRP, NC_SHARD], F32, kind="ExternalInput").ap()
    out_d = nc.dram_tensor("out", [M, NC_SHARD], F32, kind="ExternalOutput").ap()

    # psum free-dim chunks (bank = 512 f32)
    CHUNKS = [(c, min(512, NC_SHARD - c)) for c in range(0, NC_SHARD, 512)]

    with tile.TileContext(nc) as tc:
        import contextlib

        ctx = contextlib.ExitStack()
        with ctx:
            consts = ctx.enter_context(tc.tile_pool(name="consts", bufs=1))
            wpool = ctx.enter_context(tc.tile_pool(name="w", bufs=1))
            wqld = ctx.enter_context(tc.tile_pool(name="wqld", bufs=2))
            wsp = ctx.enter_context(tc.tile_pool(name="ws", bufs=2))
            wsb = ctx.enter_context(tc.tile_pool(name="wsb", bufs=2))
            xp = ctx.enter_context(tc.tile_pool(name="x", bufs=2))
            up = ctx.enter_context(tc.tile_pool(name="u", bufs=1))
            ap_ = ctx.enter_context(tc.tile_pool(name="a", bufs=2))
            atp = ctx.enter_context(tc.tile_pool(name="at", bufs=1))
            outp = ctx.enter_context(tc.tile_pool(name="o", bufs=2))
            vecs = ctx.enter_context(tc.tile_pool(name="v", bufs=2))
            ps_out = ctx.enter_context(
                tc.tile_pool(name="pso", bufs=2, space="PSUM"))
            ps_tr = ctx.enter_context(
                tc.tile_pool(name="pst", bufs=2, space="PSUM"))

            ident = consts.tile([128, 128], BF16)
            make_identity(nc, ident[:])
            magic_vec = consts.tile([128, 1], F32)
            nc.vector.memset(magic_vec[:], MAGIC)



            # W holds all dequantized weights, k-major: [k%128, g, n]
            W = wpool.tile([128, NGRP * NC_SHARD], BF16)

            def wslice(g, c0, cw):
                return W[:, g * NC_SHARD + c0: g * NC_SHARD + c0 + cw]

            # ---- per-m-tile phases ----
            x_tiles = {}
            scale_of = {}
            at_of = {}

            def quant_phase(m):
                x_t = xp.tile([128, K], F32, tag="xt")
                nc.sync.dma_start(x_t[:], x_d[m * 128:(m + 1) * 128, :])
                mx = vecs.tile([128, 1], F32, tag="mx")
                nc.vector.tensor_reduce(mx[:], x_t[:], axis=AX.X, op=ALU.max)
                mn = vecs.tile([128, 1], F32, tag="mn")
                nc.vector.tensor_reduce(mn[:], x_t[:], axis=AX.X, op=ALU.min)
                xc = vecs.tile([128, 1], F32, tag="xc")
                nc.vector.tensor_scalar(xc[:], mx[:], 0.0, None, ALU.max)
                nn_ = vecs.tile([128, 1], F32, tag="nn")
                nc.vector.tensor_scalar(nn_[:], mn[:], 0.0, None, ALU.min)
                df = vecs.tile([128, 1], F32, tag="df")
                nc.vector.tensor_tensor(df[:], xc[:], nn_[:], ALU.subtract)
                sc = vecs.tile([128, 1], F32, tag="sc")
                nc.vector.tensor_scalar(sc[:], df[:], 1.0 / 255.0, 1e-9,
                                        ALU.mult, ALU.max)
                inv = vecs.tile([128, 1], F32, tag="inv")
                nc.vector.reciprocal(inv[:], sc[:])
                z0 = vecs.tile([128, 1], F32, tag="z0")
                nc.vector.tensor_tensor(z0[:], nn_[:], inv[:], ALU.mult)
                z1 = vecs.tile([128, 1], F32, tag="z1")
                nc.vector.tensor_scalar(z1[:], z0[:], -1.0, -128.0,
                                        ALU.mult, ALU.add)
                zr = vecs.tile([128, 1], F32, tag="zr")
                nc.vector.tensor_scalar(zr[:], z1[:], MAGIC, MAGIC,
                                        ALU.add, ALU.subtract)
                loM = vecs.tile([128, 1], F32, tag="loM")
                nc.vector.tensor_scalar(loM[:], zr[:], -1.0, MAGIC - 128.0,
                                        ALU.mult, ALU.add)
                hiM = vecs.tile([128, 1], F32, tag="hiM")
                nc.vector.tensor_scalar(hiM[:], zr[:], -1.0, MAGIC + 127.0,
                                        ALU.mult, ALU.add)
                # u = x*inv + MAGIC  (ACT; the add rounds to integer, RNE)
                u = up.tile([128, K], F32, tag="u")
                nc.scalar.activation(u[:], x_t[:], ACTF.Identity,
                                     bias=magic_vec[:], scale=inv[:])
                # clip in magic space
                nc.vector.tensor_scalar(u[:], u[:], loM[:], hiM[:],
                                        ALU.max, ALU.min)
                # a = u - MAGIC -> bf16 (integer in [-255, 255], exact)
                a_t = ap_.tile([128, K], BF16, tag="a")
                nc.vector.tensor_scalar(a_t[:], u[:], MAGIC, None, ALU.subtract)
                x_tiles[m] = x_t
                scale_of[m] = sc
                at_of[m] = a_t

            def transpose_phase(m):
                a_t = at_of[m]
                aT = atp.tile([128, NGRP * 128], BF16, tag="aT")
                for q in range(NGRP // 4):
                    tr = ps_tr.tile([128, 512], BF16, tag="tr")
                    for j in range(4):
                        g = q * 4 + j
                        nc.tensor.transpose(
                            tr[:, j * 128:(j + 1) * 128],
                            a_t[:, g * 128:(g + 1) * 128], ident[:])
                    nc.scalar.copy(aT[:, q * 512:(q + 1) * 512], tr[:])
                return aT

            def matmul_phase(m, aT):
                psum = ps_out.tile([128, NC_SHARD], F32, tag="psum")
                for g in range(NGRP):
                    for (c0, cw) in CHUNKS:
                        nc.tensor.matmul(
                            psum[:, c0:c0 + cw],
                            lhsT=aT[:, g * 128:(g + 1) * 128],
                            rhs=wslice(g, c0, cw),
                            start=(g == 0), stop=(g == NGRP - 1))
                o_t = outp.tile([128, NC_SHARD], F32, tag="o")
                nc.scalar.activation(o_t[:], psum[:], ACTF.Identity,
                                     bias=0.0, scale=scale_of[m][:])
                nc.sync.dma_start(out_d[m * 128:(m + 1) * 128, :], o_t[:])

            # Emission order tuned for pipeline fill: quant m0/m1 first so
            # DVE/ACT produce aT early, then weight dequant streams, then
            # the PE phases.
            quant_phase(0)
            quant_phase(1)

            for g in range(NGRP):
                wq_t = wqld.tile([128, NC_SHARD], F32, tag="wq")
                nc.sync.dma_start(wq_t[:], wq_d[g * 128:(g + 1) * 128, :])
                ws_row = wsp.tile([1, NC_SHARD], F32, tag="wsrow")
                nc.sync.dma_start(ws_row[:], ws_d[g:g + 1, :])
                ws_bc = wsb.tile([128, NC_SHARD], F32, tag="wsb")
                nc.gpsimd.partition_broadcast(ws_bc[:], ws_row[:])
                nc.vector.tensor_tensor(W[:, g * NC_SHARD:(g + 1) * NC_SHARD],
                                        wq_t[:], ws_bc[:], ALU.mult)

            for m in range(MTILES):
                if m >= 2:
                    quant_phase(m)
                aT = transpose_phase(m)
                matmul_phase(m, aT)

    nc.compile()
    _CACHE["nc"] = nc
    return nc


def kernel(x, weight_qvals, weight_scales, group_size):
    global LAST_RESULTS
    _install_axon_ntff_hook()
    from concourse.bass_utils import run_bass_kernel_spmd

    x = np.asarray(x, dtype=np.float32)
    wq = np.asarray(weight_qvals)
    ws = np.asarray(weight_scales, dtype=np.float32)
    assert int(group_size) == GS
    assert x.shape == (M, K) and wq.shape == (N, K) and ws.shape == (N, NGRP)

    nc = _build()

    in_maps = []
    for c in range(NCORES):
        sl = slice(c * NC_SHARD, (c + 1) * NC_SHARD)
        wq_c = np.ascontiguousarray(wq[sl].T).astype(np.float32)
        ws_c = np.ascontiguousarray(ws[sl].T)
        in_maps.append({"x": x, "wq": wq_c, "ws": ws_c})

    res = run_bass_kernel_spmd(nc, in_maps, core_ids=list(range(NCORES)))
    LAST_RESULTS = res
    out = np.concatenate([r["out"] for r in res.results], axis=1)
    return out


if __name__ == "__main__":
    rng = np.random.default_rng(0)
    xv = rng.standard_normal((M, K)).astype(np.float32)
    wqv = rng.integers(-4, 4, (N, K)).astype(np.int32)
    wsv = (rng.random((N, NGRP)).astype(np.float32) * 0.02 + 1e-4)
    o = kernel(xv, wqv, wsv, GS)
    print("out shape:", o.shape, "finite:", np.isfinite(o).all())


# revision 13
# speedup vs baseline: 2.5266x; 1.2275x over previous
"""Trainium2 Bass kernel for Chn8ActGrp3WgtQuantizedLinear.

Computes: out = fake_quant8_per_row(x) @ dequant(weight_qvals, weight_scales).T

  x:             (1024, 4096)  f32
  weight_qvals:  (11008, 4096) int32, 3-bit values in [-4, 3]
  weight_scales: (11008, 32)   f32, one scale per (out-channel, 128-group)
  out:           (1024, 11008) f32

Strategy (tensor parallel over 8 NeuronCores):
  - shard N=11008 output channels -> 1376 per core; replicate x
  - host repacks weights K-major as fp16: wq[4096,1376] (3-bit values exact),
    ws[32,1376] fp16 (scales in [1e-4, 0.02] are fp16-normal; 2^-11 rounding)
  - device per core:
      * dequant W[k,n] = wq * ws[g(k),n] -> fp16 (gpsimd partition_broadcast
        of the scale row + DVE tensor_tensor, 2x mode on 16-bit dtypes)
      * per-row activation stats (min/max over 4 k-chunks as DMA lands),
        scale/zero-point; fake-quant via one ACT op (x*inv_s + MAGIC rounds
        to integer RNE) + DVE clip in magic space + ACT subtract -> a fp16
        (integer in [-255,255], exact)
      * transpose a via PE transpose-mode into aT[k,m] tiles
      * matmul: psum[m=128, n] += aT[k,m].T @ W[k,n] over 32 k-groups;
        m-tiles 0+1 run a fused loop sharing the dequant stream so the PE
        stays dense while DVE produces W
      * evict with per-row scale: out = psum * scale[m]  (ACT)
  - host concatenates the 8 (1024, 1376) shards.
"""

import os
import sys
import types

import numpy as np
import ml_dtypes

M, K, N, GS = 1024, 4096, 11008, 128
NCORES = 8
NC_SHARD = N // NCORES  # 1376
NGRP = K // GS  # 32
MTILES = M // 128  # 8
XCHUNK = 1024  # x load/reduce chunk along K
NXC = K // XCHUNK
MAGIC = 12582912.0  # 1.5 * 2**23: adding then subtracting rounds f32 to int (RNE)

_CACHE = {}
LAST_RESULTS = None


def _install_axon_ntff_hook():
    """Register the NTFF profile hook if the container's antenv lacks it.

    Only needed for trace=True (BASS_TRACE=1); degrades silently."""
    try:
        if "antenv.axon_hooks" in sys.modules:
            return
        import antenv

        mod = types.ModuleType("antenv.axon_hooks")
        _state = {"hook": None}
        mod.set_axon_ntff_profile_hook = lambda h: _state.__setitem__("hook", h)
        mod.get_axon_ntff_profile_hook = lambda: _state["hook"]
        sys.modules["antenv.axon_hooks"] = mod
        antenv.axon_hooks = mod

        from trn_agent_boot.trn_boot import _ntff_profile_via_ctypes

        mod.set_axon_ntff_profile_hook(
            _ntff_profile_via_ctypes("/opt/axon/libaxon_pjrt.so")
        )
    except Exception:
        pass


def _build():
    if "nc" in _CACHE:
        return _CACHE["nc"]

    import concourse.bass as bass
    import concourse.tile as tile
    from concourse import bacc, mybir
    from concourse.masks import make_identity

    dt = mybir.dt
    F32, F16 = dt.float32, dt.float16
    ALU = mybir.AluOpType
    ACTF = mybir.ActivationFunctionType
    AX = mybir.AxisListType

    nc = bacc.Bacc("TRN2", target_bir_lowering=False, debug=False,
                   num_devices=NCORES)

    x_d = nc.dram_tensor("x", [M, K], F32, kind="ExternalInput").ap()
    wq_d = nc.dram_tensor("wq", [K, NC_SHARD], F16, kind="ExternalInput").ap()
    ws_d = nc.dram_tensor("ws", [NGRP, NC_SHARD], F16, kind="ExternalInput").ap()
    out_d = nc.dram_tensor("out", [M, NC_SHARD], F32, kind="ExternalOutput").ap()

    CHUNKS = [(c, min(512, NC_SHARD - c)) for c in range(0, NC_SHARD, 512)]

    with tile.TileContext(nc) as tc:
        import contextlib

        ctx = contextlib.ExitStack()
        with ctx:
            consts = ctx.enter_context(tc.tile_pool(name="consts", bufs=1))
            wpool = ctx.enter_context(tc.tile_pool(name="w", bufs=1))
            wqld = ctx.enter_context(tc.tile_pool(name="wqld", bufs=3))
            wsp = ctx.enter_context(tc.tile_pool(name="ws", bufs=3))
            wsb = ctx.enter_context(tc.tile_pool(name="wsb", bufs=3))
            xp = ctx.enter_context(tc.tile_pool(name="x", bufs=2))
            up = ctx.enter_context(tc.tile_pool(name="u", bufs=1))
            ap_ = ctx.enter_context(tc.tile_pool(name="a", bufs=2))
            atp = ctx.enter_context(tc.tile_pool(name="at", bufs=2))
            outp = ctx.enter_context(tc.tile_pool(name="o", bufs=2))
            vecs = ctx.enter_context(tc.tile_pool(name="v", bufs=2))
            ps_out = ctx.enter_context(
                tc.tile_pool(name="pso", bufs=2, space="PSUM"))
            ps_tr = ctx.enter_context(
                tc.tile_pool(name="pst", bufs=2, space="PSUM"))

            ident = consts.tile([128, 128], F16)
            make_identity(nc, ident[:])
            magic_vec = consts.tile([128, 1], F32)
            nc.vector.memset(magic_vec[:], MAGIC)
            neg_magic_vec = consts.tile([128, 1], F32)
            nc.vector.memset(neg_magic_vec[:], -MAGIC)

            # W holds all dequantized weights, k-major: [k%128, g, n]
            W = wpool.tile([128, NGRP * NC_SHARD], F16)

            scale_of = {}
            at_of = {}

            def quant_phase(m):
                """x load (chunked) -> row stats -> fake-quant -> a fp16."""
                x_t = xp.tile([128, K], F32, tag="xt")
                mxp = vecs.tile([128, NXC], F32, tag="mxp")
                mnp = vecs.tile([128, NXC], F32, tag="mnp")
                for j in range(NXC):
                    sl = slice(j * XCHUNK, (j + 1) * XCHUNK)
                    nc.sync.dma_start(x_t[:, sl], x_d[m * 128:(m + 1) * 128, sl])
                    nc.vector.tensor_reduce(mxp[:, j:j + 1], x_t[:, sl],
                                            axis=AX.X, op=ALU.max)
                    nc.vector.tensor_reduce(mnp[:, j:j + 1], x_t[:, sl],
                                            axis=AX.X, op=ALU.min)
                mx = vecs.tile([128, 1], F32, tag="mx")
                nc.vector.tensor_reduce(mx[:], mxp[:], axis=AX.X, op=ALU.max)
                mn = vecs.tile([128, 1], F32, tag="mn")
                nc.vector.tensor_reduce(mn[:], mnp[:], axis=AX.X, op=ALU.min)
                xc = vecs.tile([128, 1], F32, tag="xc")
                nc.vector.tensor_scalar(xc[:], mx[:], 0.0, None, ALU.max)
                nn_ = vecs.tile([128, 1], F32, tag="nn")
                nc.vector.tensor_scalar(nn_[:], mn[:], 0.0, None, ALU.min)
                df = vecs.tile([128, 1], F32, tag="df")
                nc.vector.tensor_tensor(df[:], xc[:], nn_[:], ALU.subtract)
                sc = vecs.tile([128, 1], F32, tag="sc")
                nc.vector.tensor_scalar(sc[:], df[:], 1.0 / 255.0, 1e-9,
                                        ALU.mult, ALU.max)
                inv = vecs.tile([128, 1], F32, tag="inv")
                nc.vector.reciprocal(inv[:], sc[:])
                z0 = vecs.tile([128, 1], F32, tag="z0")
                nc.vector.tensor_tensor(z0[:], nn_[:], inv[:], ALU.mult)
                z1 = vecs.tile([128, 1], F32, tag="z1")
                nc.vector.tensor_scalar(z1[:], z0[:], -1.0, -128.0,
                                        ALU.mult, ALU.add)
                zr = vecs.tile([128, 1], F32, tag="zr")
                nc.vector.tensor_scalar(zr[:], z1[:], MAGIC, MAGIC,
                                        ALU.add, ALU.subtract)
                loM = vecs.tile([128, 1], F32, tag="loM")
                nc.vector.tensor_scalar(loM[:], zr[:], -1.0, MAGIC - 128.0,
                                        ALU.mult, ALU.add)
                hiM = vecs.tile([128, 1], F32, tag="hiM")
                nc.vector.tensor_scalar(hiM[:], zr[:], -1.0, MAGIC + 127.0,
                                        ALU.mult, ALU.add)
                # u = x*inv + MAGIC  (ACT; the add rounds to integer, RNE)
                u = up.tile([128, K], F32, tag="u")
                nc.scalar.activation(u[:], x_t[:], ACTF.Identity,
                                     bias=magic_vec[:], scale=inv[:])
                # clip in magic space (DVE, fp32 tensor_scalar 2x mode)
                nc.vector.tensor_scalar(u[:], u[:], loM[:], hiM[:],
                                        ALU.max, ALU.min)
                # a = u - MAGIC -> fp16 (integer in [-255, 255], exact) on ACT
                a_t = ap_.tile([128, K], F16, tag="a")
                nc.scalar.activation(a_t[:], u[:], ACTF.Identity,
                                     bias=neg_magic_vec[:], scale=1.0)
                scale_of[m] = sc
                at_of[m] = a_t

            def transpose_phase(m):
                a_t = at_of[m]
                aT = atp.tile([128, NGRP * 128], F16, tag="aT")
                for q in range(NGRP // 4):
                    tr = ps_tr.tile([128, 512], F16, tag="tr")
                    for j in range(4):
                        g = q * 4 + j
                        nc.tensor.transpose(
                            tr[:, j * 128:(j + 1) * 128],
                            a_t[:, g * 128:(g + 1) * 128], ident[:])
                    nc.scalar.copy(aT[:, q * 512:(q + 1) * 512], tr[:])
                return aT

            def evict_phase(m, psum):
                o_t = outp.tile([128, NC_SHARD], F32, tag="o")
                nc.scalar.activation(o_t[:], psum[:], ACTF.Identity,
                                     bias=0.0, scale=scale_of[m][:])
                nc.sync.dma_start(out_d[m * 128:(m + 1) * 128, :], o_t[:])

            # ---- emission (DVE priority order: quant m0, m1, dequant, rest)
            quant_phase(0)
            quant_phase(1)

            for g in range(NGRP):
                wq_t = wqld.tile([128, NC_SHARD], F16, tag="wq")
                nc.sync.dma_start(wq_t[:], wq_d[g * 128:(g + 1) * 128, :])
                ws_row = wsp.tile([1, NC_SHARD], F16, tag="wsrow")
                # scalar-engine HWDGE queue: keeps tiny row DMAs off the
                # sync queue that carries the big x/wq/out transfers
                nc.scalar.dma_start(ws_row[:], ws_d[g:g + 1, :])
                ws_bc = wsb.tile([128, NC_SHARD], F16, tag="wsb")
                nc.gpsimd.partition_broadcast(ws_bc[:], ws_row[:])
                nc.vector.tensor_tensor(W[:, g * NC_SHARD:(g + 1) * NC_SHARD],
                                        wq_t[:], ws_bc[:], ALU.mult)

            # fused m0+m1 matmul phase: both consume each W group as it lands
            aT0 = transpose_phase(0)
            aT1 = transpose_phase(1)
            ps0 = ps_out.tile([128, NC_SHARD], F32, tag="psum")
            ps1 = ps_out.tile([128, NC_SHARD], F32, tag="psum")
            for g in range(NGRP):
                for (c0, cw) in CHUNKS:
                    nc.tensor.matmul(ps0[:, c0:c0 + cw],
                                     lhsT=aT0[:, g * 128:(g + 1) * 128],
                                     rhs=W[:, g * NC_SHARD + c0:
                                           g * NC_SHARD + c0 + cw],
                                     start=(g == 0), stop=(g == NGRP - 1))
                for (c0, cw) in CHUNKS:
                    nc.tensor.matmul(ps1[:, c0:c0 + cw],
                                     lhsT=aT1[:, g * 128:(g + 1) * 128],
                                     rhs=W[:, g * NC_SHARD + c0:
                                           g * NC_SHARD + c0 + cw],
                                     start=(g == 0), stop=(g == NGRP - 1))
            evict_phase(0, ps0)
            evict_phase(1, ps1)

            for m in range(2, MTILES):
                quant_phase(m)
                aT = transpose_phase(m)
                psum = ps_out.tile([128, NC_SHARD], F32, tag="psum")
                for g in range(NGRP):
                    for (c0, cw) in CHUNKS:
                        nc.tensor.matmul(psum[:, c0:c0 + cw],
                                         lhsT=aT[:, g * 128:(g + 1) * 128],
                                         rhs=W[:, g * NC_SHARD + c0:
                                               g * NC_SHARD + c0 + cw],
                                         start=(g == 0), stop=(g == NGRP - 1))
                evict_phase(m, psum)

    nc.compile()
    _CACHE["nc"] = nc
    return nc


def kernel(x, weight_qvals, weight_scales, group_size):
    global LAST_RESULTS
    _install_axon_ntff_hook()
    from concourse.bass_utils import run_bass_kernel_spmd

    x = np.asarray(x, dtype=np.float32)
    wq = np.asarray(weight_qvals)
    ws = np.asarray(weight_scales, dtype=np.float32)
    assert int(group_size) == GS
    assert x.shape == (M, K) and wq.shape == (N, K) and ws.shape == (N, NGRP)

    nc = _build()

    in_maps = []
    for c in range(NCORES):
        sl = slice(c * NC_SHARD, (c + 1) * NC_SHARD)
        wq_c = np.ascontiguousarray(wq[sl].T).astype(np.float16)
        ws_c = np.ascontiguousarray(ws[sl].T).astype(np.float16)
        in_maps.append({"x": x, "wq": wq_c, "ws": ws_c})

    res = run_bass_kernel_spmd(nc, in_maps, core_ids=list(range(NCORES)))
    LAST_RESULTS = res
    out = np.concatenate([r["out"] for r in res.results], axis=1)
    return out


if __name__ == "__main__":
    rng = np.random.default_rng(0)
    xv = rng.standard_normal((M, K)).astype(np.float32)
    wqv = rng.integers(-4, 4, (N, K)).astype(np.int32)
    wsv = (rng.random((N, NGRP)).astype(np.float32) * 0.02 + 1e-4)
    o = kernel(xv, wqv, wsv, GS)
    print("out shape:", o.shape, "finite:", np.isfinite(o).all())


# revision 23
# speedup vs baseline: 2.5523x; 1.0102x over previous
"""Trainium2 Bass kernel for Chn8ActGrp3WgtQuantizedLinear.

Computes: out = fake_quant8_per_row(x) @ dequant(weight_qvals, weight_scales).T

  x:             (1024, 4096)  f32
  weight_qvals:  (11008, 4096) int32, 3-bit values in [-4, 3]
  weight_scales: (11008, 32)   f32, one scale per (out-channel, 128-group)
  out:           (1024, 11008) f32

Strategy (tensor parallel over 8 NeuronCores):
  - shard N=11008 output channels -> 1376 per core; replicate x
  - host repacks weights K-major as fp16: wq[4096,1376] (3-bit values exact),
    ws[32,1376] fp16 (scales in [1e-4, 0.02] are fp16-normal; 2^-11 rounding)
  - device per core:
      * dequant W[k,n] = wq * ws[g(k),n] -> fp16 (gpsimd partition_broadcast
        of the scale row + DVE tensor_tensor, 2x mode on 16-bit dtypes)
      * per-row activation stats (min/max over 4 k-chunks as DMA lands),
        scale/zero-point; fake-quant via one ACT op (x*inv_s + MAGIC rounds
        to integer RNE) + DVE clip in magic space + ACT subtract -> a fp16
        (integer in [-255,255], exact)
      * transpose a via PE transpose-mode into aT[k,m] tiles
      * matmul: psum[m=128, n] += aT[k,m].T @ W[k,n] over 32 k-groups;
        m-tiles 0+1 run a fused loop sharing the dequant stream so the PE
        stays dense while DVE produces W
      * evict with per-row scale: out = psum * scale[m]  (ACT)
  - host concatenates the 8 (1024, 1376) shards.
"""

import os
import sys
import types

import numpy as np
import ml_dtypes

M, K, N, GS = 1024, 4096, 11008, 128
NCORES = 8
NC_SHARD = N // NCORES  # 1376
NGRP = K // GS  # 32
MTILES = M // 128  # 8
XCHUNK = 1024  # x load/reduce chunk along K
NXC = K // XCHUNK
MAGIC = 12582912.0  # 1.5 * 2**23: adding then subtracting rounds f32 to int (RNE)

_CACHE = {}
LAST_RESULTS = None


def _install_axon_ntff_hook():
    """Register the NTFF profile hook if the container's antenv lacks it.

    Only needed for trace=True (BASS_TRACE=1); degrades silently."""
    try:
        if "antenv.axon_hooks" in sys.modules:
            return
        import antenv

        mod = types.ModuleType("antenv.axon_hooks")
        _state = {"hook": None}
        mod.set_axon_ntff_profile_hook = lambda h: _state.__setitem__("hook", h)
        mod.get_axon_ntff_profile_hook = lambda: _state["hook"]
        sys.modules["antenv.axon_hooks"] = mod
        antenv.axon_hooks = mod

        from trn_agent_boot.trn_boot import _ntff_profile_via_ctypes

        mod.set_axon_ntff_profile_hook(
            _ntff_profile_via_ctypes("/opt/axon/libaxon_pjrt.so")
        )
    except Exception:
        pass


def _build():
    if "nc" in _CACHE:
        return _CACHE["nc"]

    import concourse.bass as bass
    import concourse.tile as tile
    from concourse import bacc, mybir
    from concourse.masks import make_identity

    dt = mybir.dt
    F32, F16 = dt.float32, dt.float16
    ALU = mybir.AluOpType
    ACTF = mybir.ActivationFunctionType
    AX = mybir.AxisListType

    nc = bacc.Bacc("TRN2", target_bir_lowering=False, debug=False,
                   num_devices=NCORES)

    x_d = nc.dram_tensor("x", [M, K], F32, kind="ExternalInput").ap()
    wq_d = nc.dram_tensor("wq", [K, NC_SHARD], F16, kind="ExternalInput").ap()
    # scale rows batched 2 groups per row for single-call partition_broadcast
    ws_d = nc.dram_tensor("ws", [NGRP // 2, 2 * NC_SHARD], F16,
                          kind="ExternalInput").ap()
    out_d = nc.dram_tensor("out", [M, NC_SHARD], F32, kind="ExternalOutput").ap()

    CHUNKS = [(c, min(512, NC_SHARD - c)) for c in range(0, NC_SHARD, 512)]

    with tile.TileContext(nc) as tc:
        import contextlib

        ctx = contextlib.ExitStack()
        with ctx:
            consts = ctx.enter_context(tc.tile_pool(name="consts", bufs=1))
            wpool = ctx.enter_context(tc.tile_pool(name="w", bufs=1))
            wqld = ctx.enter_context(tc.tile_pool(name="wqld", bufs=3))
            wsp = ctx.enter_context(tc.tile_pool(name="ws", bufs=2))
            wsb = ctx.enter_context(tc.tile_pool(name="wsb", bufs=2))
            xp = ctx.enter_context(tc.tile_pool(name="x", bufs=2))
            up = ctx.enter_context(tc.tile_pool(name="u", bufs=1))
            ap_ = ctx.enter_context(tc.tile_pool(name="a", bufs=2))
            atp = ctx.enter_context(tc.tile_pool(name="at", bufs=2))
            outp = ctx.enter_context(tc.tile_pool(name="o", bufs=2))
            vecs = ctx.enter_context(tc.tile_pool(name="v", bufs=2))
            ps_out = ctx.enter_context(
                tc.tile_pool(name="pso", bufs=2, space="PSUM"))
            ps_tr = ctx.enter_context(
                tc.tile_pool(name="pst", bufs=2, space="PSUM"))

            ident = consts.tile([128, 128], F16)
            make_identity(nc, ident[:])
            magic_vec = consts.tile([128, 1], F32)
            nc.vector.memset(magic_vec[:], MAGIC)
            neg_magic_vec = consts.tile([128, 1], F32)
            nc.vector.memset(neg_magic_vec[:], -MAGIC)

            # W holds all dequantized weights, k-major: [k%128, g, n]
            W = wpool.tile([128, NGRP * NC_SHARD], F16)

            scale_of = {}
            at_of = {}

            def quant_phase(m):
                """x load (chunked) -> row stats -> fake-quant -> a fp16."""
                x_t = xp.tile([128, K], F32, tag="xt")
                mxp = vecs.tile([128, NXC], F32, tag="mxp")
                mnp = vecs.tile([128, NXC], F32, tag="mnp")
                for j in range(NXC):
                    sl = slice(j * XCHUNK, (j + 1) * XCHUNK)
                    nc.sync.dma_start(x_t[:, sl], x_d[m * 128:(m + 1) * 128, sl])
                    nc.vector.tensor_reduce(mxp[:, j:j + 1], x_t[:, sl],
                                            axis=AX.X, op=ALU.max)
                    nc.vector.tensor_reduce(mnp[:, j:j + 1], x_t[:, sl],
                                            axis=AX.X, op=ALU.min)
                mx = vecs.tile([128, 1], F32, tag="mx")
                nc.vector.tensor_reduce(mx[:], mxp[:], axis=AX.X, op=ALU.max)
                mn = vecs.tile([128, 1], F32, tag="mn")
                nc.vector.tensor_reduce(mn[:], mnp[:], axis=AX.X, op=ALU.min)
                xc = vecs.tile([128, 1], F32, tag="xc")
                nc.vector.tensor_scalar(xc[:], mx[:], 0.0, None, ALU.max)
                nn_ = vecs.tile([128, 1], F32, tag="nn")
                nc.vector.tensor_scalar(nn_[:], mn[:], 0.0, None, ALU.min)
                df = vecs.tile([128, 1], F32, tag="df")
                nc.vector.tensor_tensor(df[:], xc[:], nn_[:], ALU.subtract)
                sc = vecs.tile([128, 1], F32, tag="sc")
                nc.vector.tensor_scalar(sc[:], df[:], 1.0 / 255.0, 1e-9,
                                        ALU.mult, ALU.max)
                inv = vecs.tile([128, 1], F32, tag="inv")
                nc.vector.reciprocal(inv[:], sc[:])
                z0 = vecs.tile([128, 1], F32, tag="z0")
                nc.vector.tensor_tensor(z0[:], nn_[:], inv[:], ALU.mult)
                z1 = vecs.tile([128, 1], F32, tag="z1")
                nc.vector.tensor_scalar(z1[:], z0[:], -1.0, -128.0,
                                        ALU.mult, ALU.add)
                zr = vecs.tile([128, 1], F32, tag="zr")
                nc.vector.tensor_scalar(zr[:], z1[:], MAGIC, MAGIC,
                                        ALU.add, ALU.subtract)
                loM = vecs.tile([128, 1], F32, tag="loM")
                nc.vector.tensor_scalar(loM[:], zr[:], -1.0, MAGIC - 128.0,
                                        ALU.mult, ALU.add)
                hiM = vecs.tile([128, 1], F32, tag="hiM")
                nc.vector.tensor_scalar(hiM[:], zr[:], -1.0, MAGIC + 127.0,
                                        ALU.mult, ALU.add)
                # u = x*inv + MAGIC  (the add rounds to integer, RNE)
                u = up.tile([128, K], F32, tag="u")
                nc.vector.tensor_scalar(u[:], x_t[:], inv[:], MAGIC,
                                        ALU.mult, ALU.add)
                # clip in magic space (DVE, fp32 tensor_scalar 2x mode)
                nc.vector.tensor_scalar(u[:], u[:], loM[:], hiM[:],
                                        ALU.max, ALU.min)
                # a = u - MAGIC -> fp16 (integer in [-255, 255], exact) on ACT
                a_t = ap_.tile([128, K], F16, tag="a")
                nc.scalar.activation(a_t[:], u[:], ACTF.Identity,
                                     bias=neg_magic_vec[:], scale=1.0)
                scale_of[m] = sc
                at_of[m] = a_t

            def transpose_phase(m):
                a_t = at_of[m]
                aT = atp.tile([128, NGRP * 128], F16, tag="aT")
                for q in range(NGRP // 4):
                    tr = ps_tr.tile([128, 512], F16, tag="tr")
                    for j in range(4):
                        g = q * 4 + j
                        nc.tensor.transpose(
                            tr[:, j * 128:(j + 1) * 128],
                            a_t[:, g * 128:(g + 1) * 128], ident[:])
                    nc.scalar.copy(aT[:, q * 512:(q + 1) * 512], tr[:])
                return aT

            def evict_phase(m, psum):
                o_t = outp.tile([128, NC_SHARD], F32, tag="o")
                nc.scalar.activation(o_t[:], psum[:], ACTF.Identity,
                                     bias=0.0, scale=scale_of[m][:])
                nc.sync.dma_start(out_d[m * 128:(m + 1) * 128, :], o_t[:])

            # ---- emission (DVE priority order: quant m0, m1, dequant, rest)
            quant_phase(0)
            quant_phase(1)

            for b in range(NGRP // 2):
                ws_row = wsp.tile([1, 2 * NC_SHARD], F16, tag="wsrow")
                # scalar-engine HWDGE queue: keeps tiny row DMAs off the
                # sync queue that carries the big x/wq/out transfers
                nc.scalar.dma_start(ws_row[:], ws_d[b:b + 1, :])
                ws_bc = wsb.tile([128, 2 * NC_SHARD], F16, tag="wsb")
                nc.gpsimd.partition_broadcast(ws_bc[:], ws_row[:])
                for j in range(2):
                    g = b * 2 + j
                    wq_t = wqld.tile([128, NC_SHARD], F16, tag="wq")
                    nc.sync.dma_start(wq_t[:], wq_d[g * 128:(g + 1) * 128, :])
                    nc.vector.tensor_tensor(
                        W[:, g * NC_SHARD:(g + 1) * NC_SHARD], wq_t[:],
                        ws_bc[:, j * NC_SHARD:(j + 1) * NC_SHARD], ALU.mult)

            # fused m0+m1 matmul phase: both consume each W group as it
            # lands. m0 runs g0..3 solo so its MMs start before m1's
            # transposes block the in-order PE queue.
            aT0 = transpose_phase(0)
            ps0 = ps_out.tile([128, NC_SHARD], F32, tag="psum")
            ps1 = ps_out.tile([128, NC_SHARD], F32, tag="psum")

            def mm_group(psum, aT, g):
                for (c0, cw) in CHUNKS:
                    nc.tensor.matmul(psum[:, c0:c0 + cw],
                                     lhsT=aT[:, g * 128:(g + 1) * 128],
                                     rhs=W[:, g * NC_SHARD + c0:
                                           g * NC_SHARD + c0 + cw],
                                     start=(g == 0), stop=(g == NGRP - 1))

            for g in range(4):
                mm_group(ps0, aT0, g)
            aT1 = transpose_phase(1)
            for g in range(4):
                mm_group(ps1, aT1, g)
            for g in range(4, NGRP):
                mm_group(ps0, aT0, g)
                mm_group(ps1, aT1, g)
            evict_phase(0, ps0)
            evict_phase(1, ps1)

            for m in range(2, MTILES):
                quant_phase(m)
                aT = transpose_phase(m)
                psum = ps_out.tile([128, NC_SHARD], F32, tag="psum")
                for g in range(NGRP):
                    for (c0, cw) in CHUNKS:
                        nc.tensor.matmul(psum[:, c0:c0 + cw],
                                         lhsT=aT[:, g * 128:(g + 1) * 128],
                                         rhs=W[:, g * NC_SHARD + c0:
                                               g * NC_SHARD + c0 + cw],
                                         start=(g == 0), stop=(g == NGRP - 1))
                evict_phase(m, psum)

    nc.compile()
    _CACHE["nc"] = nc
    return nc


def kernel(x, weight_qvals, weight_scales, group_size):
    global LAST_RESULTS
    _install_axon_ntff_hook()
    from concourse.bass_utils import run_bass_kernel_spmd

    x = np.asarray(x, dtype=np.float32)
    wq = np.asarray(weight_qvals)
    ws = np.asarray(weight_scales, dtype=np.float32)
    assert int(group_size) == GS
    assert x.shape == (M, K) and wq.shape == (N, K) and ws.shape == (N, NGRP)

    nc = _build()

    in_maps = []
    for c in range(NCORES):
        sl = slice(c * NC_SHARD, (c + 1) * NC_SHARD)
        wq_c = np.ascontiguousarray(wq[sl].T).astype(np.float16)
        ws_c = np.ascontiguousarray(ws[sl].T).astype(np.float16).reshape(
            NGRP // 2, 2 * NC_SHARD)
        in_maps.append({"x": x, "wq": wq_c, "ws": ws_c})

    res = run_bass_kernel_spmd(nc, in_maps, core_ids=list(range(NCORES)))
    LAST_RESULTS = res
    out = np.concatenate([r["out"] for r in res.results], axis=1)
    return out


if __name__ == "__main__":
    rng = np.random.default_rng(0)
    xv = rng.standard_normal((M, K)).astype(np.float32)
    wqv = rng.integers(-4, 4, (N, K)).astype(np.int32)
    wsv = (rng.random((N, NGRP)).astype(np.float32) * 0.02 + 1e-4)
    o = kernel(xv, wqv, wsv, GS)
    print("out shape:", o.shape, "finite:", np.isfinite(o).all())


# revision 26
# speedup vs baseline: 2.7408x; 1.0739x over previous
"""Trainium2 Bass kernel for Chn8ActGrp3WgtQuantizedLinear.

Computes: out = fake_quant8_per_row(x) @ dequant(weight_qvals, weight_scales).T

  x:             (1024, 4096)  f32
  weight_qvals:  (11008, 4096) int32, 3-bit values in [-4, 3]
  weight_scales: (11008, 32)   f32, one scale per (out-channel, 128-group)
  out:           (1024, 11008) f32

Strategy (tensor parallel over 8 NeuronCores):
  - shard N=11008 output channels -> 1376 per core; replicate x
  - host repacks weights K-major as fp16: wq[4096,1376] (3-bit values exact),
    ws[32,1376] fp16 (scales in [1e-4, 0.02] are fp16-normal; 2^-11 rounding)
  - device per core:
      * dequant W[k,n] = wq * ws[g(k),n] -> fp16 (gpsimd partition_broadcast
        of the scale row + DVE tensor_tensor, 2x mode on 16-bit dtypes)
      * per-row activation stats (min/max over 4 k-chunks as DMA lands),
        scale/zero-point; fake-quant via one ACT op (x*inv_s + MAGIC rounds
        to integer RNE) + DVE clip in magic space + ACT subtract -> a fp16
        (integer in [-255,255], exact)
      * transpose a via PE transpose-mode into aT[k,m] tiles
      * matmul: psum[m=128, n] += aT[k,m].T @ W[k,n] over 32 k-groups;
        m-tiles 0+1 run a fused loop sharing the dequant stream so the PE
        stays dense while DVE produces W
      * evict with per-row scale: out = psum * scale[m]  (ACT)
  - host concatenates the 8 (1024, 1376) shards.
"""

import os
import sys
import types

import numpy as np
import ml_dtypes

M, K, N, GS = 1024, 4096, 11008, 128
NCORES = 8
NC_SHARD = N // NCORES  # 1376
NGRP = K // GS  # 32
MTILES = M // 128  # 8
XCHUNK = 1024  # x load/reduce chunk along K
NXC = K // XCHUNK
MAGIC = 12582912.0  # 1.5 * 2**23: adding then subtracting rounds f32 to int (RNE)

_CACHE = {}
LAST_RESULTS = None


def _install_axon_ntff_hook():
    """Register the NTFF profile hook if the container's antenv lacks it.

    Only needed for trace=True (BASS_TRACE=1); degrades silently."""
    try:
        if "antenv.axon_hooks" in sys.modules:
            return
        import antenv

        mod = types.ModuleType("antenv.axon_hooks")
        _state = {"hook": None}
        mod.set_axon_ntff_profile_hook = lambda h: _state.__setitem__("hook", h)
        mod.get_axon_ntff_profile_hook = lambda: _state["hook"]
        sys.modules["antenv.axon_hooks"] = mod
        antenv.axon_hooks = mod

        from trn_agent_boot.trn_boot import _ntff_profile_via_ctypes

        mod.set_axon_ntff_profile_hook(
            _ntff_profile_via_ctypes("/opt/axon/libaxon_pjrt.so")
        )
    except Exception:
        pass


def _build():
    if "nc" in _CACHE:
        return _CACHE["nc"]

    import concourse.bass as bass
    import concourse.tile as tile
    from concourse import bacc, mybir
    from concourse.masks import make_identity

    dt = mybir.dt
    F32, F16 = dt.float32, dt.float16
    ALU = mybir.AluOpType
    ACTF = mybir.ActivationFunctionType
    AX = mybir.AxisListType

    nc = bacc.Bacc("TRN2", target_bir_lowering=False, debug=False,
                   num_devices=NCORES)

    x_d = nc.dram_tensor("x", [M, K], F32, kind="ExternalInput").ap()
    wq_d = nc.dram_tensor("wq", [K, NC_SHARD], F16, kind="ExternalInput").ap()
    # scales pre-broadcast on host: block b = groups (2b, 2b+1), 128 rows
    ws_d = nc.dram_tensor("ws", [(NGRP // 2) * 128, 2 * NC_SHARD], F16,
                          kind="ExternalInput").ap()
    out_d = nc.dram_tensor("out", [M, NC_SHARD], F32, kind="ExternalOutput").ap()

    CHUNKS = [(c, min(512, NC_SHARD - c)) for c in range(0, NC_SHARD, 512)]

    with tile.TileContext(nc) as tc:
        import contextlib

        ctx = contextlib.ExitStack()
        with ctx:
            consts = ctx.enter_context(tc.tile_pool(name="consts", bufs=1))
            wpool = ctx.enter_context(tc.tile_pool(name="w", bufs=1))
            wqld = ctx.enter_context(tc.tile_pool(name="wqld", bufs=3))
            wsp = ctx.enter_context(tc.tile_pool(name="ws", bufs=2))
            wsb = ctx.enter_context(tc.tile_pool(name="wsb", bufs=2))
            xp = ctx.enter_context(tc.tile_pool(name="x", bufs=2))
            up = ctx.enter_context(tc.tile_pool(name="u", bufs=1))
            ap_ = ctx.enter_context(tc.tile_pool(name="a", bufs=2))
            atp = ctx.enter_context(tc.tile_pool(name="at", bufs=2))
            outp = ctx.enter_context(tc.tile_pool(name="o", bufs=2))
            vecs = ctx.enter_context(tc.tile_pool(name="v", bufs=2))
            ps_out = ctx.enter_context(
                tc.tile_pool(name="pso", bufs=2, space="PSUM"))
            ps_tr = ctx.enter_context(
                tc.tile_pool(name="pst", bufs=2, space="PSUM"))

            ident = consts.tile([128, 128], F16)
            make_identity(nc, ident[:])
            magic_vec = consts.tile([128, 1], F32)
            nc.vector.memset(magic_vec[:], MAGIC)
            neg_magic_vec = consts.tile([128, 1], F32)
            nc.vector.memset(neg_magic_vec[:], -MAGIC)

            # W holds all dequantized weights, k-major: [k%128, g, n]
            W = wpool.tile([128, NGRP * NC_SHARD], F16)

            scale_of = {}
            at_of = {}

            def quant_phase(m):
                """x load (chunked) -> row stats -> fake-quant -> a fp16."""
                x_t = xp.tile([128, K], F32, tag="xt")
                mxp = vecs.tile([128, NXC], F32, tag="mxp")
                mnp = vecs.tile([128, NXC], F32, tag="mnp")
                for j in range(NXC):
                    sl = slice(j * XCHUNK, (j + 1) * XCHUNK)
                    nc.sync.dma_start(x_t[:, sl], x_d[m * 128:(m + 1) * 128, sl])
                    nc.vector.tensor_reduce(mxp[:, j:j + 1], x_t[:, sl],
                                            axis=AX.X, op=ALU.max)
                    nc.vector.tensor_reduce(mnp[:, j:j + 1], x_t[:, sl],
                                            axis=AX.X, op=ALU.min)
                mx = vecs.tile([128, 1], F32, tag="mx")
                nc.vector.tensor_reduce(mx[:], mxp[:], axis=AX.X, op=ALU.max)
                mn = vecs.tile([128, 1], F32, tag="mn")
                nc.vector.tensor_reduce(mn[:], mnp[:], axis=AX.X, op=ALU.min)
                xc = vecs.tile([128, 1], F32, tag="xc")
                nc.vector.tensor_scalar(xc[:], mx[:], 0.0, None, ALU.max)
                nn_ = vecs.tile([128, 1], F32, tag="nn")
                nc.vector.tensor_scalar(nn_[:], mn[:], 0.0, None, ALU.min)
                df = vecs.tile([128, 1], F32, tag="df")
                nc.vector.tensor_tensor(df[:], xc[:], nn_[:], ALU.subtract)
                sc = vecs.tile([128, 1], F32, tag="sc")
                nc.vector.tensor_scalar(sc[:], df[:], 1.0 / 255.0, 1e-9,
                                        ALU.mult, ALU.max)
                inv = vecs.tile([128, 1], F32, tag="inv")
                nc.vector.reciprocal(inv[:], sc[:])
                z0 = vecs.tile([128, 1], F32, tag="z0")
                nc.vector.tensor_tensor(z0[:], nn_[:], inv[:], ALU.mult)
                z1 = vecs.tile([128, 1], F32, tag="z1")
                nc.vector.tensor_scalar(z1[:], z0[:], -1.0, -128.0,
                                        ALU.mult, ALU.add)
                zr = vecs.tile([128, 1], F32, tag="zr")
                nc.vector.tensor_scalar(zr[:], z1[:], MAGIC, MAGIC,
                                        ALU.add, ALU.subtract)
                loM = vecs.tile([128, 1], F32, tag="loM")
                nc.vector.tensor_scalar(loM[:], zr[:], -1.0, MAGIC - 128.0,
                                        ALU.mult, ALU.add)
                hiM = vecs.tile([128, 1], F32, tag="hiM")
                nc.vector.tensor_scalar(hiM[:], zr[:], -1.0, MAGIC + 127.0,
                                        ALU.mult, ALU.add)
                # u = x*inv + MAGIC  (the add rounds to integer, RNE)
                u = up.tile([128, K], F32, tag="u")
                nc.vector.tensor_scalar(u[:], x_t[:], inv[:], MAGIC,
                                        ALU.mult, ALU.add)
                # clip in magic space (DVE, fp32 tensor_scalar 2x mode)
                nc.vector.tensor_scalar(u[:], u[:], loM[:], hiM[:],
                                        ALU.max, ALU.min)
                # a = u - MAGIC -> fp16 (integer in [-255, 255], exact) on ACT
                a_t = ap_.tile([128, K], F16, tag="a")
                nc.scalar.activation(a_t[:], u[:], ACTF.Identity,
                                     bias=neg_magic_vec[:], scale=1.0)
                scale_of[m] = sc
                at_of[m] = a_t

            def transpose_phase(m):
                a_t = at_of[m]
                aT = atp.tile([128, NGRP * 128], F16, tag="aT")
                for q in range(NGRP // 4):
                    tr = ps_tr.tile([128, 512], F16, tag="tr")
                    for j in range(4):
                        g = q * 4 + j
                        nc.tensor.transpose(
                            tr[:, j * 128:(j + 1) * 128],
                            a_t[:, g * 128:(g + 1) * 128], ident[:])
                    nc.scalar.copy(aT[:, q * 512:(q + 1) * 512], tr[:])
                return aT

            def evict_phase(m, psum):
                o_t = outp.tile([128, NC_SHARD], F32, tag="o")
                nc.scalar.activation(o_t[:], psum[:], ACTF.Identity,
                                     bias=0.0, scale=scale_of[m][:])
                nc.sync.dma_start(out_d[m * 128:(m + 1) * 128, :], o_t[:])

            # ---- emission (DVE priority order: quant m0, m1, dequant, rest)
            quant_phase(0)
            quant_phase(1)

            for b in range(NGRP // 2):
                ws_bc = wsb.tile([128, 2 * NC_SHARD], F16, tag="wsb")
                # scalar-engine HWDGE queue: keeps these off the sync queue
                nc.scalar.dma_start(ws_bc[:], ws_d[b * 128:(b + 1) * 128, :])
                for j in range(2):
                    g = b * 2 + j
                    wq_t = wqld.tile([128, NC_SHARD], F16, tag="wq")
                    nc.sync.dma_start(wq_t[:], wq_d[g * 128:(g + 1) * 128, :])
                    nc.vector.tensor_tensor(
                        W[:, g * NC_SHARD:(g + 1) * NC_SHARD], wq_t[:],
                        ws_bc[:, j * NC_SHARD:(j + 1) * NC_SHARD], ALU.mult)

            # fused m0+m1 matmul phase: both consume each W group as it
            # lands. m0 runs g0..3 solo so its MMs start before m1's
            # transposes block the in-order PE queue.
            aT0 = transpose_phase(0)
            ps0 = ps_out.tile([128, NC_SHARD], F32, tag="psum")
            ps1 = ps_out.tile([128, NC_SHARD], F32, tag="psum")

            def mm_group(psum, aT, g):
                for (c0, cw) in CHUNKS:
                    nc.tensor.matmul(psum[:, c0:c0 + cw],
                                     lhsT=aT[:, g * 128:(g + 1) * 128],
                                     rhs=W[:, g * NC_SHARD + c0:
                                           g * NC_SHARD + c0 + cw],
                                     start=(g == 0), stop=(g == NGRP - 1))

            for g in range(4):
                mm_group(ps0, aT0, g)
            aT1 = transpose_phase(1)
            for g in range(4):
                mm_group(ps1, aT1, g)
            for g in range(4, NGRP):
                mm_group(ps0, aT0, g)
                mm_group(ps1, aT1, g)
            evict_phase(0, ps0)
            evict_phase(1, ps1)

            for m in range(2, MTILES):
                quant_phase(m)
                aT = transpose_phase(m)
                psum = ps_out.tile([128, NC_SHARD], F32, tag="psum")
                for g in range(NGRP):
                    for (c0, cw) in CHUNKS:
                        nc.tensor.matmul(psum[:, c0:c0 + cw],
                                         lhsT=aT[:, g * 128:(g + 1) * 128],
                                         rhs=W[:, g * NC_SHARD + c0:
                                               g * NC_SHARD + c0 + cw],
                                         start=(g == 0), stop=(g == NGRP - 1))
                evict_phase(m, psum)

    nc.compile()
    _CACHE["nc"] = nc
    return nc


def kernel(x, weight_qvals, weight_scales, group_size):
    global LAST_RESULTS
    _install_axon_ntff_hook()
    from concourse.bass_utils import run_bass_kernel_spmd

    x = np.asarray(x, dtype=np.float32)
    wq = np.asarray(weight_qvals)
    ws = np.asarray(weight_scales, dtype=np.float32)
    assert int(group_size) == GS
    assert x.shape == (M, K) and wq.shape == (N, K) and ws.shape == (N, NGRP)

    nc = _build()

    in_maps = []
    for c in range(NCORES):
        sl = slice(c * NC_SHARD, (c + 1) * NC_SHARD)
        wq_c = np.ascontiguousarray(wq[sl].T).astype(np.float16)
        ws_c = np.ascontiguousarray(
            np.broadcast_to(
                ws[sl].T.astype(np.float16).reshape(NGRP // 2, 1, 2 * NC_SHARD),
                (NGRP // 2, 128, 2 * NC_SHARD),
            )
        ).reshape((NGRP // 2) * 128, 2 * NC_SHARD)
        in_maps.append({"x": x, "wq": wq_c, "ws": ws_c})

    res = run_bass_kernel_spmd(nc, in_maps, core_ids=list(range(NCORES)))
    LAST_RESULTS = res
    out = np.concatenate([r["out"] for r in res.results], axis=1)
    return out


if __name__ == "__main__":
    rng = np.random.default_rng(0)
    xv = rng.standard_normal((M, K)).astype(np.float32)
    wqv = rng.integers(-4, 4, (N, K)).astype(np.int32)
    wsv = (rng.random((N, NGRP)).astype(np.float32) * 0.02 + 1e-4)
    o = kernel(xv, wqv, wsv, GS)
    print("out shape:", o.shape, "finite:", np.isfinite(o).all())
